# revision 12
# baseline (speedup 1.0000x reference)
"""Multi-head attention (b=2, s=2048, d_model=1024, H=16) on 8 TRN2 NeuronCores.

Head-sharded tensor parallelism: core c owns heads 2c and 2c+1 (a 128-wide
slice of the QKV feature dim). Each core computes its heads' Q/K/V, both
scores layouts (natural for softmax + attn output, transposed for the A@V
matmul), its partial x@W_O contribution, and writes its attn slice.
Host sums the 8 partial outputs and concatenates attn head slices.

Engine split: TensorE does all matmuls in bf16 (full-rate); ScalarE does
only the two exp passes (with free rowsum accumulation on the natural
layout); VectorE/GPSIMD do PSUM evacuation, softmax normalization, and
the per-head 1/rowsum-scaled W_O combine.
"""

import numpy as np
import ml_dtypes

import concourse.bass as bass
import concourse.tile as tile
from concourse import mybir
from concourse.bass import ts
from concourse.bass_utils import run_bass_kernel_spmd
from concourse.masks import make_identity

F32 = mybir.dt.float32
BF16 = mybir.dt.bfloat16
AF = mybir.ActivationFunctionType
MULT = mybir.AluOpType.mult
ADD = mybir.AluOpType.add

B = 2
S = 2048
D = 1024
H = 16
DK = 64
N_CORES = 8
T = B * S  # 4096 flattened tokens
DC = D // N_CORES  # 128 features per core (2 heads)

MAX_WAITS = 1  # this walrus accepts at most 1 sync-wait on CTRL-class ops


def _split_multi_waits(nc, max_waits=MAX_WAITS):
    """Move excess sem-waits onto preceding NoOps (walrus CTRL wait limit)."""
    ctr = [0]
    for f in nc.m.functions:
        for bb in f.blocks:
            new_insts = []
            for ins in bb.instructions:
                si = getattr(ins, "sync_info", None)
                if si is not None and len(si.on_wait) > max_waits:
                    waits = list(si.on_wait)
                    extra, keep = waits[:-max_waits], waits[-max_waits:]
                    for i in range(0, len(extra), max_waits):
                        ctr[0] += 1
                        nop = mybir.InstNoOp(
                            name=f"waitfix-{ctr[0]}", ins=[], outs=[]
                        )
                        nop.engine = ins.engine
                        nop.sync_info = mybir.SyncInfo(
                            on_wait=extra[i : i + max_waits], on_update=[]
                        )
                        nc.register_instruction(nop, overwrite=True)
                        new_insts.append(nop)
                    ins.sync_info = mybir.SyncInfo(
                        on_wait=keep, on_update=list(si.on_update)
                    )
                new_insts.append(ins)
            bb.instructions[:] = new_insts



def _av_pair(nc, V, pav, ests, b, kc, first, last=False):
    """Column-tiled A@V pair: h0 accumulates into pav[0:64], h1 into
    pav[64:128], concurrently via disjoint PE column groups."""
    g = b * 16 + kc
    est0, est1 = ests
    for qh in range(2):
        nc.tensor.matmul(
            pav[0:64, qh * 512 : (qh + 1) * 512],
            V[:, g * 128 : g * 128 + 64],
            est0[:, qh * 512 : (qh + 1) * 512],
            start=first, stop=last,
            tile_position=(0, 0),
        )
        nc.tensor.matmul(
            pav[64:128, qh * 512 : (qh + 1) * 512],
            V[:, g * 128 + 64 : g * 128 + 128],
            est1[:, qh * 512 : (qh + 1) * 512],
            start=first, stop=last,
            tile_position=(0, 64),
        )


def build_nc():
    nc = bass.Bass(trn_type="TRN2")

    xT = nc.dram_tensor("xT", [D, T], BF16, kind="ExternalInput")
    wqT = nc.dram_tensor("wqT", [D, DC], BF16, kind="ExternalInput")
    wkT = nc.dram_tensor("wkT", [D, DC], BF16, kind="ExternalInput")
    wvT = nc.dram_tensor("wvT", [D, DC], BF16, kind="ExternalInput")
    woT = nc.dram_tensor("woT", [DC, D], BF16, kind="ExternalInput")
    attn = nc.dram_tensor("attn", [B, 2, S, S], F32, kind="ExternalOutput")
    outp = nc.dram_tensor("outp", [T, D], F32, kind="ExternalOutput")

    e_np = np.zeros((2, 128), np.float32)
    e_np[0, :64] = 1.0
    e_np[1, 64:] = 1.0
    e_dram = nc.inline_tensor(e_np, name="Emat")

    with tile.TileContext(nc) as tc:
        with (
            tc.tile_pool(name="const", bufs=1) as const,
            tc.tile_pool(name="xt", bufs=2) as xtp,
            tc.tile_pool(name="p", bufs=4) as pp,
            tc.tile_pool(name="est", bufs=4) as estp,
            tc.tile_pool(name="vt", bufs=2) as vtp,
            tc.tile_pool(name="osb", bufs=4) as osbp,
            tc.tile_pool(name="small", bufs=4) as smallp,
            tc.tile_pool(name="big", bufs=2, space="PSUM") as psbig,
            tc.tile_pool(name="psav", bufs=2, space="PSUM") as psav,
        ):
            # ---- constants / persistent buffers ----
            identity = const.tile([128, 128], BF16, tag="ident")
            make_identity(nc, identity)
            ident32 = const.tile([128, 128], F32, tag="ident32")
            make_identity(nc, ident32)
            e_sb = const.tile([2, 128], F32, tag="emat")
            nc.sync.dma_start(out=e_sb, in_=e_dram[:, :])

            wq_sb = const.tile([128, 8, DC], BF16, tag="wq")
            wk_sb = const.tile([128, 8, DC], BF16, tag="wk")
            wv_sb = const.tile([128, 8, DC], BF16, tag="wv")
            for wsb, wdram in ((wq_sb, wqT), (wk_sb, wkT), (wv_sb, wvT)):
                nc.sync.dma_start(
                    out=wsb, in_=wdram.rearrange("(c p) m -> p c m", p=128)
                )
            wo_sb = const.tile([128, D], BF16, tag="wo")
            nc.sync.dma_start(out=wo_sb, in_=woT[:, :])

            QT = const.tile([128, T], BF16, tag="qt")  # scaled by 1/8
            KT = const.tile([128, T], BF16, tag="kt")
            V = const.tile([128, T], BF16, tag="v")  # block g: V[g*128+p, dc]
            houT = const.tile([128, T], BF16, tag="hout")  # [dc, t] unnormalized
            rbuf = const.tile([128, 64], F32, tag="rbuf")  # 1/rowsum, col bh*16+qt
            r2 = const.tile([2, T], F32, tag="r2")  # [local head, t] 1/rowsum

            # ---- phase A: projections QT, KT, V (+ on-chip transposes of V) ----
            xT_v = xT.rearrange("(c p) t -> p c t", p=128)
            for tb in range(8):
                xt = xtp.tile([128, 8, 512], BF16, tag="xt")
                nc.sync.dma_start(out=xt, in_=xT_v[:, :, ts(tb, 512)])
                for wsb, dest, scale in (
                    (wq_sb, QT, 0.125),
                    (wk_sb, KT, 1.0),
                ):
                    ps = psbig.tile([128, 512], F32, tag="big")
                    for ch in range(8):
                        nc.tensor.matmul(
                            ps,
                            wsb[:, ch, :],
                            xt[:, ch, :],
                            start=(ch == 0),
                            stop=(ch == 7),
                        )
                    if scale == 1.0:
                        nc.vector.tensor_copy(dest[:, ts(tb, 512)], ps)
                    else:
                        nc.vector.tensor_scalar_mul(
                            dest[:, ts(tb, 512)], ps, scale
                        )
                # V^T block then PE-transpose into natural layout
                ps = psbig.tile([128, 512], F32, tag="big")
                for ch in range(8):
                    nc.tensor.matmul(
                        ps,
                        wv_sb[:, ch, :],
                        xt[:, ch, :],
                        start=(ch == 0),
                        stop=(ch == 7),
                    )
                vt = vtp.tile([128, 512], BF16, tag="vt")
                nc.vector.tensor_copy(vt, ps)
                for j in range(4):
                    pt = psav.tile([128, 128], BF16, tag="av")
                    nc.tensor.transpose(pt, vt[:, ts(j, 128)], identity)
                    g = tb * 4 + j
                    nc.vector.tensor_copy(V[:, ts(g, 128)], pt)

            # ---- phase B: attention, both heads of a batch interleaved so
            # their K=64 matmuls land in disjoint PE row groups (h0: rows
            # 0-63, h1: rows 64-127) and execute concurrently. ----
            for b in range(2):
                t0 = b * S

                # B1: natural scores, exp+rowsum, normalize, attn output
                for qt in range(16):
                    q0 = t0 + qt * 128
                    qs0 = QT[0:64, q0 : q0 + 128]
                    qs1 = QT[64:128, q0 : q0 + 128]
                    expPh = [
                        pp.tile([128, S], F32, name=f"expP0_{b}_{qt}", tag="p"),
                        pp.tile([128, S], F32, name=f"expP1_{b}_{qt}", tag="p"),
                    ]
                    rsx = [[None, None], [None, None]]
                    for half in range(2):
                        ps0 = psbig.tile([128, 1024], F32, tag="big")
                        ps1 = psbig.tile([128, 1024], F32, tag="big")
                        for kc in range(2):
                            k0 = t0 + (half * 2 + kc) * 512
                            nc.tensor.matmul(
                                ps0[:, ts(kc, 512)], qs0,
                                KT[0:64, k0 : k0 + 512],
                                start=True, stop=True,
                            )
                            nc.tensor.matmul(
                                ps1[:, ts(kc, 512)], qs1,
                                KT[64:128, k0 : k0 + 512],
                                start=True, stop=True,
                            )
                        for hh, psx in ((0, ps0), (1, ps1)):
                            rsv = smallp.tile([128, 1], F32, tag="rs")
                            rsx[hh][half] = rsv
                            nc.scalar.activation(
                                expPh[hh][:, half * 1024 : (half + 1) * 1024],
                                psx, AF.Exp, accum_out=rsv,
                            )
                    for hh in range(2):
                        rs = smallp.tile([128, 1], F32, tag="rsum")
                        nc.vector.tensor_add(rs, rsx[hh][0], rsx[hh][1])
                        rcol = rbuf[:, (b * 2 + hh) * 16 + qt : (b * 2 + hh) * 16 + qt + 1]
                        nc.vector.reciprocal(rcol, rs)
                        nc.vector.tensor_scalar_mul(expPh[hh], expPh[hh], rcol)
                        nc.sync.dma_start(
                            out=attn[b, hh, qt * 128 : (qt + 1) * 128, :],
                            in_=expPh[hh],
                        )

                # B2: transposed scores + unnormalized A@V; both heads share
                # one PSUM accumulator via column tiling (h0 -> partitions
                # 0-63, h1 -> partitions 64-127).
                for qc in range(2):  # q-chunks of 1024
                    pav = psav.tile([128, 1024], F32, tag="av")
                    q0 = t0 + qc * 1024
                    prev = None
                    for kc in range(16):
                        k0 = t0 + kc * 128
                        pst0 = psbig.tile([128, 1024], F32, tag="big")
                        pst1 = psbig.tile([128, 1024], F32, tag="big")
                        for qh in range(2):
                            nc.tensor.matmul(
                                pst0[:, ts(qh, 512)],
                                KT[0:64, k0 : k0 + 128],
                                QT[0:64, q0 + qh * 512 : q0 + (qh + 1) * 512],
                                start=True, stop=True,
                            )
                            nc.tensor.matmul(
                                pst1[:, ts(qh, 512)],
                                KT[64:128, k0 : k0 + 128],
                                QT[64:128, q0 + qh * 512 : q0 + (qh + 1) * 512],
                                start=True, stop=True,
                            )
                        est0 = estp.tile([128, 1024], BF16, tag="est")
                        est1 = estp.tile([128, 1024], BF16, tag="est")
                        nc.scalar.activation(est0, pst0, AF.Exp)
                        nc.scalar.activation(est1, pst1, AF.Exp)
                        if prev is not None:
                            _av_pair(nc, V, pav, prev, b, kc - 1, first=(kc == 1))
                        prev = (est0, est1)
                    _av_pair(nc, V, pav, prev, b, 15, first=False, last=True)
                    nc.vector.tensor_copy(houT[:, q0 : q0 + 1024], pav)

                # both heads of batch b done: transpose 1/rowsum columns into
                # free-layout rows, broadcast across partitions via a K=2
                # matmul, normalize houT in place, then one W_O matmul per
                # output tile.
                for qt in range(16):
                    col = (b * 2) * 16 + qt
                    prt = psav.tile([2, 128], F32, tag="av")
                    nc.tensor.transpose(
                        prt, rbuf[:, col : col + 17 : 16], ident32
                    )
                    nc.vector.tensor_copy(
                        r2[0:2, t0 + qt * 128 : t0 + (qt + 1) * 128], prt
                    )
                for tch in range(4):
                    cols = slice(t0 + tch * 512, t0 + (tch + 1) * 512)
                    prb = psav.tile([128, 512], F32, tag="av")
                    nc.tensor.matmul(
                        prb, e_sb, r2[:, cols], start=True, stop=True
                    )
                    nc.vector.tensor_tensor(
                        houT[:, cols], houT[:, cols], prb, op=MULT
                    )
                for tt in range(16):
                    g = b * 16 + tt
                    for dch in range(2):
                        po = psbig.tile([128, 512], F32, tag="big")
                        nc.tensor.matmul(
                            po,
                            houT[:, ts(g, 128)],
                            wo_sb[:, ts(dch, 512)],
                            start=True, stop=True,
                        )
                        osb = osbp.tile([128, 512], F32, tag="osb")
                        nc.vector.tensor_copy(osb, po)
                        nc.sync.dma_start(
                            out=outp[
                                g * 128 : (g + 1) * 128,
                                dch * 512 : (dch + 1) * 512,
                            ],
                            in_=osb,
                        )

    _split_multi_waits(nc)
    return nc


_NC = None


def _get_nc():
    global _NC
    if _NC is None:
        _NC = build_nc()
    return _NC


def _bf16(a):
    return np.ascontiguousarray(a).astype(ml_dtypes.bfloat16)


def kernel(x, W_Q, W_K, W_V, W_O, _trace=False):
    x = np.asarray(x, dtype=np.float32)
    W_Q = np.asarray(W_Q, dtype=np.float32)
    W_K = np.asarray(W_K, dtype=np.float32)
    W_V = np.asarray(W_V, dtype=np.float32)
    W_O = np.asarray(W_O, dtype=np.float32)

    xT = _bf16(x.reshape(T, D).T)
    in_maps = []
    for c in range(N_CORES):
        sl = slice(c * DC, (c + 1) * DC)
        in_maps.append(
            {
                "xT": xT,
                "wqT": _bf16(W_Q[sl, :].T),
                "wkT": _bf16(W_K[sl, :].T),
                "wvT": _bf16(W_V[sl, :].T),
                "woT": _bf16(W_O[:, sl].T),
            }
        )

    nc = _get_nc()
    res = run_bass_kernel_spmd(
        nc, in_maps, core_ids=list(range(N_CORES)), trace=_trace
    )

    out = np.zeros((T, D), np.float32)
    attn = np.empty((B, H, S, S), np.float32)
    for c in range(N_CORES):
        out += res.results[c]["outp"]
        attn[:, 2 * c : 2 * c + 2] = res.results[c]["attn"]
    out = out.reshape(B, S, D)
    if _trace:
        return (out, attn), res
    return (out, attn)


# revision 15
# speedup vs baseline: 1.1159x; 1.1159x over previous
"""Multi-head attention (b=2, s=2048, d_model=1024, H=16) on 8 TRN2 NeuronCores.

Head-sharded tensor parallelism: core c owns heads 2c and 2c+1 (a 128-wide
slice of the QKV feature dim). Each core computes its heads' Q/K/V, both
scores layouts (natural for softmax + attn output, transposed for the A@V
matmul), its partial x@W_O contribution, and writes its attn slice.
Host sums the 8 partial outputs and concatenates attn head slices.

Engine split: TensorE does all matmuls in bf16 (full-rate); ScalarE does
only the two exp passes (with free rowsum accumulation on the natural
layout); VectorE/GPSIMD do PSUM evacuation, softmax normalization, and
the per-head 1/rowsum-scaled W_O combine.
"""

import numpy as np
import ml_dtypes

import concourse.bass as bass
import concourse.tile as tile
from concourse import mybir
from concourse.bass import ts
from concourse.bass_utils import run_bass_kernel_spmd
from concourse.masks import make_identity

F32 = mybir.dt.float32
BF16 = mybir.dt.bfloat16
AF = mybir.ActivationFunctionType
MULT = mybir.AluOpType.mult
ADD = mybir.AluOpType.add

B = 2
S = 2048
D = 1024
H = 16
DK = 64
N_CORES = 8
T = B * S  # 4096 flattened tokens
DC = D // N_CORES  # 128 features per core (2 heads)

MAX_WAITS = 1  # this walrus accepts at most 1 sync-wait on CTRL-class ops


def _split_multi_waits(nc, max_waits=MAX_WAITS):
    """Move excess sem-waits onto preceding NoOps (walrus CTRL wait limit)."""
    ctr = [0]
    for f in nc.m.functions:
        for bb in f.blocks:
            new_insts = []
            for ins in bb.instructions:
                si = getattr(ins, "sync_info", None)
                if si is not None and len(si.on_wait) > max_waits:
                    waits = list(si.on_wait)
                    extra, keep = waits[:-max_waits], waits[-max_waits:]
                    for i in range(0, len(extra), max_waits):
                        ctr[0] += 1
                        nop = mybir.InstNoOp(
                            name=f"waitfix-{ctr[0]}", ins=[], outs=[]
                        )
                        nop.engine = ins.engine
                        nop.sync_info = mybir.SyncInfo(
                            on_wait=extra[i : i + max_waits], on_update=[]
                        )
                        nc.register_instruction(nop, overwrite=True)
                        new_insts.append(nop)
                    ins.sync_info = mybir.SyncInfo(
                        on_wait=keep, on_update=list(si.on_update)
                    )
                new_insts.append(ins)
            bb.instructions[:] = new_insts



def _av_pair(nc, V, pavs, ests, b, kc, first, last=False):
    """A@V in 64x64 array mode. Contraction k-halves go to SEPARATE psum
    tiles (pavs[kh]) because concurrent row tiles must not touch the same
    PSUM bank; heads go to output column halves (h0 -> [0:64], h1 ->
    [64:128]). All four quadrant tiles run concurrently."""
    g = b * 16 + kc
    est0, est1 = ests
    for qh in range(2):
        cs = slice(qh * 512, (qh + 1) * 512)
        for kh in range(2):
            kr = slice(kh * 64, kh * 64 + 64)
            nc.tensor.matmul(
                pavs[kh][0:64, cs],
                V[kr, g * 128 : g * 128 + 64],
                est0[kr, cs],
                start=first, stop=last,
            )
            nc.tensor.matmul(
                pavs[kh][64:128, cs],
                V[kr, g * 128 + 64 : g * 128 + 128],
                est1[kr, cs],
                start=first, stop=last,
            )


def build_nc():
    nc = bass.Bass(trn_type="TRN2")

    xT = nc.dram_tensor("xT", [D, T], BF16, kind="ExternalInput")
    wqT = nc.dram_tensor("wqT", [D, DC], BF16, kind="ExternalInput")
    wkT = nc.dram_tensor("wkT", [D, DC], BF16, kind="ExternalInput")
    wvT = nc.dram_tensor("wvT", [D, DC], BF16, kind="ExternalInput")
    woT = nc.dram_tensor("woT", [DC, D], BF16, kind="ExternalInput")
    attn = nc.dram_tensor("attn", [B, 2, S, S], F32, kind="ExternalOutput")
    outp = nc.dram_tensor("outp", [T, D], F32, kind="ExternalOutput")

    e_np = np.zeros((2, 128), np.float32)
    e_np[0, :64] = 1.0
    e_np[1, 64:] = 1.0
    e_dram = nc.inline_tensor(e_np, name="Emat")

    with tile.TileContext(nc) as tc:
        with (
            tc.tile_pool(name="const", bufs=1) as const,
            tc.tile_pool(name="xt", bufs=2) as xtp,
            tc.tile_pool(name="p", bufs=4) as pp,
            tc.tile_pool(name="est", bufs=4) as estp,
            tc.tile_pool(name="vt", bufs=8) as vtp,
            tc.tile_pool(name="osb", bufs=4) as osbp,
            tc.tile_pool(name="small", bufs=4) as smallp,
            tc.tile_pool(name="big", bufs=2, space="PSUM") as psbig,
            tc.tile_pool(name="psav", bufs=2, space="PSUM") as psav,
        ):
            # ---- constants / persistent buffers ----
            identity = const.tile([128, 128], BF16, tag="ident")
            make_identity(nc, identity)
            ident32 = const.tile([128, 128], F32, tag="ident32")
            make_identity(nc, ident32)
            e_sb = const.tile([2, 128], F32, tag="emat")
            nc.sync.dma_start(out=e_sb, in_=e_dram[:, :])

            wq_sb = const.tile([128, 8, DC], BF16, tag="wq")
            wk_sb = const.tile([128, 8, DC], BF16, tag="wk")
            wv_sb = const.tile([128, 8, DC], BF16, tag="wv")
            for wsb, wdram in ((wq_sb, wqT), (wk_sb, wkT), (wv_sb, wvT)):
                nc.sync.dma_start(
                    out=wsb, in_=wdram.rearrange("(c p) m -> p c m", p=128)
                )
            wo_sb = const.tile([128, D], BF16, tag="wo")
            nc.sync.dma_start(out=wo_sb, in_=woT[:, :])

            QT = const.tile([128, T], BF16, tag="qt")  # scaled by 1/8
            KT = const.tile([128, T], BF16, tag="kt")
            V = const.tile([128, T], BF16, tag="v")  # block g: V[g*128+p, dc]
            houT = const.tile([128, T], BF16, tag="hout")  # [dc, t] unnormalized
            rbuf = const.tile([128, 64], F32, tag="rbuf")  # 1/rowsum, col bh*16+qt
            r2 = const.tile([2, T], F32, tag="r2")  # [local head, t] 1/rowsum

            # ---- phase A: projections QT, KT, V (+ on-chip transposes of V) ----
            xT_v = xT.rearrange("(c p) t -> p c t", p=128)
            vts = []
            for tb in range(8):
                xt = xtp.tile([128, 8, 512], BF16, tag="xt")
                nc.sync.dma_start(out=xt, in_=xT_v[:, :, ts(tb, 512)])
                for wsb, dest, scale in (
                    (wq_sb, QT, 0.125),
                    (wk_sb, KT, 1.0),
                ):
                    ps = psbig.tile([128, 512], F32, tag="big")
                    for ch in range(8):
                        nc.tensor.matmul(
                            ps,
                            wsb[:, ch, :],
                            xt[:, ch, :],
                            start=(ch == 0),
                            stop=(ch == 7),
                        )
                    if scale == 1.0:
                        nc.vector.tensor_copy(dest[:, ts(tb, 512)], ps)
                    else:
                        nc.vector.tensor_scalar_mul(
                            dest[:, ts(tb, 512)], ps, scale
                        )
                # V^T block then PE-transpose into natural layout
                ps = psbig.tile([128, 512], F32, tag="big")
                for ch in range(8):
                    nc.tensor.matmul(
                        ps,
                        wv_sb[:, ch, :],
                        xt[:, ch, :],
                        start=(ch == 0),
                        stop=(ch == 7),
                    )
                vt = vtp.tile([128, 512], BF16, name=f"vt_{tb}", tag="vt")
                nc.vector.tensor_copy(vt, ps)
                vts.append(vt)
            # all V^T blocks done: one transpose-mode stretch for the whole V
            for tb in range(8):
                for j in range(4):
                    pt = psav.tile([128, 128], BF16, tag="av")
                    nc.tensor.transpose(pt, vts[tb][:, ts(j, 128)], identity)
                    g = tb * 4 + j
                    nc.vector.tensor_copy(V[:, ts(g, 128)], pt)

            # ---- phase B: attention, both heads of a batch interleaved so
            # their K=64 matmuls land in disjoint PE row groups (h0: rows
            # 0-63, h1: rows 64-127) and execute concurrently. ----
            for b in range(2):
                t0 = b * S

                # B1: natural scores, exp+rowsum, normalize, attn output
                for qt in range(16):
                    q0 = t0 + qt * 128
                    expPh = [
                        pp.tile([128, S], F32, name=f"expP0_{b}_{qt}", tag="p"),
                        pp.tile([128, S], F32, name=f"expP1_{b}_{qt}", tag="p"),
                    ]
                    rsx = [[None, None], [None, None]]
                    for half in range(2):
                        ps0 = psbig.tile([128, 1024], F32, tag="big")
                        ps1 = psbig.tile([128, 1024], F32, tag="big")
                        for kc in range(2):
                            k0 = t0 + (half * 2 + kc) * 512
                            for ph, psx, mh in (
                                (0, ps0, 0), (0, ps0, 1),
                                (1, ps1, 0), (1, ps1, 1),
                            ):
                                hr = slice(ph * 64, ph * 64 + 64)
                                nc.tensor.matmul(
                                    psx[mh * 64 : mh * 64 + 64, ts(kc, 512)],
                                    QT[hr, q0 + mh * 64 : q0 + (mh + 1) * 64],
                                    KT[hr, k0 : k0 + 512],
                                    start=True, stop=True,
                                )
                        for hh, psx in ((0, ps0), (1, ps1)):
                            rsv = smallp.tile([128, 1], F32, tag="rs")
                            rsx[hh][half] = rsv
                            nc.scalar.activation(
                                expPh[hh][:, half * 1024 : (half + 1) * 1024],
                                psx, AF.Exp, accum_out=rsv,
                            )
                    for hh in range(2):
                        rs = smallp.tile([128, 1], F32, tag="rsum")
                        nc.vector.tensor_add(rs, rsx[hh][0], rsx[hh][1])
                        rcol = rbuf[:, (b * 2 + hh) * 16 + qt : (b * 2 + hh) * 16 + qt + 1]
                        nc.vector.reciprocal(rcol, rs)
                        nc.vector.tensor_scalar_mul(expPh[hh], expPh[hh], rcol)
                        nc.sync.dma_start(
                            out=attn[b, hh, qt * 128 : (qt + 1) * 128, :],
                            in_=expPh[hh],
                        )

                # B2: transposed scores + unnormalized A@V; both heads share
                # one PSUM accumulator via column tiling (h0 -> partitions
                # 0-63, h1 -> partitions 64-127).
                for qc in range(2):  # q-chunks of 1024
                    pav_k0 = psav.tile([128, 1024], F32, tag="av")
                    pav_k1 = psav.tile([128, 1024], F32, tag="av")
                    pavs = (pav_k0, pav_k1)
                    q0 = t0 + qc * 1024
                    prev = None
                    for kc in range(16):
                        k0 = t0 + kc * 128
                        pst0 = psbig.tile([128, 1024], F32, tag="big")
                        pst1 = psbig.tile([128, 1024], F32, tag="big")
                        for qh in range(2):
                            for ph, psx in ((0, pst0), (1, pst1)):
                                hr = slice(ph * 64, ph * 64 + 64)
                                for mh in range(2):
                                    nc.tensor.matmul(
                                        psx[mh * 64 : mh * 64 + 64, ts(qh, 512)],
                                        KT[hr, k0 + mh * 64 : k0 + (mh + 1) * 64],
                                        QT[hr, q0 + qh * 512 : q0 + (qh + 1) * 512],
                                        start=True, stop=True,
                                    )
                        est0 = estp.tile([128, 1024], BF16, tag="est")
                        est1 = estp.tile([128, 1024], BF16, tag="est")
                        nc.scalar.activation(est0, pst0, AF.Exp)
                        nc.scalar.activation(est1, pst1, AF.Exp)
                        if prev is not None:
                            _av_pair(nc, V, pavs, prev, b, kc - 1, first=(kc == 1))
                        prev = (est0, est1)
                    _av_pair(nc, V, pavs, prev, b, 15, first=False, last=True)
                    avtmp = osbp.tile([128, 1024], F32, name=f"avtmp_{b}_{qc}", tag="osb")
                    nc.vector.tensor_copy(avtmp, pav_k1)
                    nc.vector.tensor_tensor(
                        houT[:, q0 : q0 + 1024], pav_k0, avtmp, op=ADD
                    )

                # both heads of batch b done: transpose 1/rowsum columns into
                # free-layout rows, broadcast across partitions via a K=2
                # matmul, normalize houT in place, then one W_O matmul per
                # output tile.
                for qt in range(16):
                    col = (b * 2) * 16 + qt
                    prt = psav.tile([2, 128], F32, tag="av")
                    nc.tensor.transpose(
                        prt, rbuf[:, col : col + 17 : 16], ident32
                    )
                    nc.vector.tensor_copy(
                        r2[0:2, t0 + qt * 128 : t0 + (qt + 1) * 128], prt
                    )
                for tch in range(4):
                    cols = slice(t0 + tch * 512, t0 + (tch + 1) * 512)
                    prb = psav.tile([128, 512], F32, tag="av")
                    nc.tensor.matmul(
                        prb, e_sb, r2[:, cols], start=True, stop=True
                    )
                    nc.vector.tensor_tensor(
                        houT[:, cols], houT[:, cols], prb, op=MULT
                    )
                for tt in range(16):
                    g = b * 16 + tt
                    for dch in range(2):
                        po = psbig.tile([128, 512], F32, tag="big")
                        nc.tensor.matmul(
                            po,
                            houT[:, ts(g, 128)],
                            wo_sb[:, ts(dch, 512)],
                            start=True, stop=True,
                        )
                        osb = osbp.tile([128, 512], F32, tag="osb")
                        nc.vector.tensor_copy(osb, po)
                        nc.sync.dma_start(
                            out=outp[
                                g * 128 : (g + 1) * 128,
                                dch * 512 : (dch + 1) * 512,
                            ],
                            in_=osb,
                        )

    _split_multi_waits(nc)
    return nc


_NC = None


def _get_nc():
    global _NC
    if _NC is None:
        _NC = build_nc()
    return _NC


def _bf16(a):
    return np.ascontiguousarray(a).astype(ml_dtypes.bfloat16)


def kernel(x, W_Q, W_K, W_V, W_O, _trace=False):
    x = np.asarray(x, dtype=np.float32)
    W_Q = np.asarray(W_Q, dtype=np.float32)
    W_K = np.asarray(W_K, dtype=np.float32)
    W_V = np.asarray(W_V, dtype=np.float32)
    W_O = np.asarray(W_O, dtype=np.float32)

    xT = _bf16(x.reshape(T, D).T)
    in_maps = []
    for c in range(N_CORES):
        sl = slice(c * DC, (c + 1) * DC)
        in_maps.append(
            {
                "xT": xT,
                "wqT": _bf16(W_Q[sl, :].T),
                "wkT": _bf16(W_K[sl, :].T),
                "wvT": _bf16(W_V[sl, :].T),
                "woT": _bf16(W_O[:, sl].T),
            }
        )

    nc = _get_nc()
    res = run_bass_kernel_spmd(
        nc, in_maps, core_ids=list(range(N_CORES)), trace=_trace
    )

    out = np.zeros((T, D), np.float32)
    attn = np.empty((B, H, S, S), np.float32)
    for c in range(N_CORES):
        out += res.results[c]["outp"]
        attn[:, 2 * c : 2 * c + 2] = res.results[c]["attn"]
    out = out.reshape(B, S, D)
    if _trace:
        return (out, attn), res
    return (out, attn)
